# revision 8
# baseline (speedup 1.0000x reference)
"""Trainium2 Bass kernel for the gated-attention module (8 NeuronCores, SPMD).

Module math (per reference):
    qsig = sigmoid(qs); ksig = sigmoid(ks_p)
    vsig = sigmoid(f)*tanh(c),  (c,f) = split(sigmoid(vs) @ vq_w.T + vq_b)
    q = qsig * LN(query @ ql_w.T + ql_b)        [S,B,H]
    k = ksig * key ; v = vsig * value
    out[q,b,:] = softmax(q_h . k_h / sqrt(H)) @ v_h   (per head h)

Kernel strategy:
  - Shard (batch, query-block): core = b*4 + qc handles query rows
    [qc*512:(qc+1)*512] of batch b, with full K/V for that batch.
  - Host-side constant folding of the tiny gate vectors (pure functions of
    the module *parameters*, no data dependence):
        G  = qsig*ksig*ln_g/sqrt(H); Bv = qsig*ksig*ln_b/sqrt(H); vsig
    so on-device  q_eff = norm(y)*G + Bv,  scores = q_eff . key  (no key
    gating needed),  out = vsig * (P @ value).
  - bf16 matmul operands, pre-transposed on host into the contraction
    layouts the PE needs (q^T, k^T, w^T); fp32 psum accumulation for the
    q_linear and PV reductions; fp32 LN statistics and output.
  - Scores are computed transposed (k on partitions) so softmax's P feeds
    the PV matmul directly with no P transpose; the softmax denominator
    comes from a ones-column appended to V. exp() needs no max-subtract:
    |scores| <~ 0.4 (LN output scaled by sigmoid-gates/32), far from
    overflow.
  - Heads are processed in pairs with score matmuls interleaved at lhsT
    base-partitions 0/64 so the PE row-groups run them concurrently
    (contract dim is only 64).
"""

import sys

sys.path.insert(0, "/opt/trn_rl_repo")

import numpy as np
import ml_dtypes

S = 2048
B = 2
H = 1024
H2 = 2 * H
NH = 16
HD = 64
TQ = S // 4  # 512 query rows per core
NKC = S // 128  # 16 k-chunks
SCALE = float(np.sqrt(H))
EPS = 1e-12

_CACHE = {}


def _build_bass():
    import concourse.bacc as bacc
    import concourse.bass as bass
    import concourse.tile as tile
    from concourse import mybir
    from concourse.masks import make_identity

    f32 = mybir.dt.float32
    bf16 = mybir.dt.bfloat16
    AF = mybir.ActivationFunctionType
    ALU = mybir.AluOpType

    nc = bacc.Bacc(None, target_bir_lowering=False)

    qt_d = nc.dram_tensor("qt", [H2, TQ], bf16, kind="ExternalInput")
    kt_d = nc.dram_tensor("kt", [H, S], bf16, kind="ExternalInput")
    wt_d = nc.dram_tensor("wt", [H2, H], bf16, kind="ExternalInput")
    v_d = nc.dram_tensor("vaug", [NKC, 128, NH, HD + 1], bf16, kind="ExternalInput")
    qlb_d = nc.dram_tensor("qlb", [H], f32, kind="ExternalInput")
    g_d = nc.dram_tensor("gvec", [H], f32, kind="ExternalInput")
    bv_d = nc.dram_tensor("bvec", [H], f32, kind="ExternalInput")
    vs_d = nc.dram_tensor("vsig", [H], f32, kind="ExternalInput")
    out_d = nc.dram_tensor("out", [TQ, H], f32, kind="ExternalOutput")

    def bcast(dram_handle):
        # replicate a [H] dram vector across all 128 partitions
        ap = dram_handle[:]
        return bass.AP(tensor=ap.tensor, offset=ap.offset, ap=[[0, 128], [1, H]])

    with tile.TileContext(nc) as tc:
        with tc.tile_pool(name="persist", bufs=1) as persist:
            id_bf = persist.tile([128, 128], bf16)
            make_identity(nc, id_bf)
            id_f32 = persist.tile([128, 128], f32)
            make_identity(nc, id_f32)
            eps_t = persist.tile([128, 1], f32)
            nc.vector.memset(eps_t[:], EPS)

            qlb_r = persist.tile([128, H], f32)
            g_r = persist.tile([128, H], f32)
            bv_r = persist.tile([128, H], f32)
            vs_r = persist.tile([128, H], f32)
            nc.gpsimd.dma_start(out=qlb_r[:], in_=bcast(qlb_d))
            nc.gpsimd.dma_start(out=g_r[:], in_=bcast(g_d))
            nc.gpsimd.dma_start(out=bv_r[:], in_=bcast(bv_d))
            nc.gpsimd.dma_start(out=vs_r[:], in_=bcast(vs_d))

            # K^T tiles: kt_sb[p, dc, :] = key[:, dc*128+p] (host pre-transposed)
            kt_sb = persist.tile([128, 8, S], bf16)
            for half in range(2):
                nc.gpsimd.dma_start(
                    out=kt_sb[:, half * 4 : (half + 1) * 4, :],
                    in_=kt_d[half * 512 : (half + 1) * 512, :].rearrange(
                        "(dc p) k -> p dc k", p=128
                    ),
                )

            # V (+ ones column): vsb[p, kc, h, m] = vaug[kc, p, h, m]
            vsb = persist.tile([128, NKC, NH, HD + 1], bf16)
            nc.gpsimd.dma_start(out=vsb[:], in_=v_d[:].rearrange("c p h m -> p c h m"))

            # q_eff^T lives here: [o partitions, o-chunk, t]
            qeT = persist.tile([128, 8, TQ], bf16)
            # final output staging, one tile per 128-row query block
            outsb = [
                persist.tile([128, H], f32, name=f"outsb{i}", tag=f"outsb{i}")
                for i in range(4)
            ]

            # ---------------- phase 1+2: q_linear + LayerNorm ----------------
            with tc.tile_pool(name="ph2", bufs=1) as ph2:
                qt_sb = ph2.tile([128, 16, TQ], bf16)
                wt_sb = ph2.tile([128, 16, H], bf16)
                for g4 in range(4):
                    nc.sync.dma_start(
                        out=qt_sb[:, g4 * 4 : (g4 + 1) * 4, :],
                        in_=qt_d[g4 * 512 : (g4 + 1) * 512, :].rearrange(
                            "(ic p) t -> p ic t", p=128
                        ),
                    )
                    nc.sync.dma_start(
                        out=wt_sb[:, g4 * 4 : (g4 + 1) * 4, :],
                        in_=wt_d[g4 * 512 : (g4 + 1) * 512, :].rearrange(
                            "(ic p) o -> p ic o", p=128
                        ),
                    )
                ysb = [
                    ph2.tile([128, H], f32, name=f"ysb{i}", tag=f"ysb{i}")
                    for i in range(4)
                ]
                mv = [
                    ph2.tile([128, 2], f32, name=f"mv{i}", tag=f"mv{i}")
                    for i in range(4)
                ]
                rst = [
                    ph2.tile([128, 1], f32, name=f"rst{i}", tag=f"rst{i}")
                    for i in range(4)
                ]

                with (
                    tc.tile_pool(name="ylin", bufs=2, space="PSUM") as ylin,
                    tc.tile_pool(name="tpq", bufs=2, space="PSUM") as tpq,
                    tc.tile_pool(name="st", bufs=4) as st_pool,
                    tc.tile_pool(name="qe", bufs=2) as qe_pool,
                ):
                    for tc4 in range(4):
                        y_ps = ylin.tile([128, 2, 512], f32)
                        for ic in range(16):
                            lhsT = qt_sb[:, ic, tc4 * 128 : (tc4 + 1) * 128]
                            for oc in range(2):
                                nc.tensor.matmul(
                                    y_ps[:, oc, :],
                                    lhsT=lhsT,
                                    rhs=wt_sb[:, ic, oc * 512 : (oc + 1) * 512],
                                    start=(ic == 0),
                                    stop=(ic == 15),
                                )
                        nc.vector.tensor_add(
                            ysb[tc4][:],
                            y_ps[:].rearrange("p a b -> p (a b)"),
                            qlb_r[:],
                        )
                        st = st_pool.tile([128, 2, 6], f32)
                        nc.vector.bn_stats(st[:, 0, :], ysb[tc4][:, 0:512])
                        nc.vector.bn_stats(st[:, 1, :], ysb[tc4][:, 512:1024])
                        nc.vector.bn_aggr(mv[tc4][:], st[:])
                    # batched rstd: all Ln, then all Exp (one ACT table set)
                    lv = [
                        st_pool.tile([128, 1], f32, name=f"lv{i}", tag=f"lv{i}")
                        for i in range(4)
                    ]
                    for tc4 in range(4):
                        nc.scalar.activation(
                            lv[tc4][:], mv[tc4][:, 1:2], AF.Ln, bias=eps_t[:]
                        )
                    for tc4 in range(4):
                        nc.scalar.activation(
                            rst[tc4][:], lv[tc4][:], AF.Exp, scale=-0.5
                        )
                    for tc4 in range(4):
                        nc.vector.tensor_scalar(
                            out=ysb[tc4][:],
                            in0=ysb[tc4][:],
                            scalar1=mv[tc4][:, 0:1],
                            scalar2=rst[tc4][:],
                            op0=ALU.subtract,
                            op1=ALU.mult,
                        )
                        nc.vector.tensor_mul(ysb[tc4][:], ysb[tc4][:], g_r[:])
                        qe = qe_pool.tile([128, H], bf16)
                        nc.vector.tensor_add(qe[:], ysb[tc4][:], bv_r[:])
                        for oc8 in range(8):
                            tp = tpq.tile([128, 128], bf16)
                            nc.tensor.transpose(
                                tp[:], qe[:, oc8 * 128 : (oc8 + 1) * 128], id_bf[:]
                            )
                            nc.vector.tensor_copy(
                                qeT[:, oc8, tc4 * 128 : (tc4 + 1) * 128], tp[:]
                            )

            # ---------------- phase 3: attention, head pairs ----------------
            with (
                tc.tile_pool(name="sc", bufs=1, space="PSUM") as sc_pool,
                tc.tile_pool(name="pv", bufs=1, space="PSUM") as pv_pool,
                tc.tile_pool(name="tp2", bufs=2, space="PSUM") as tp2_pool,
                tc.tile_pool(name="pt", bufs=3) as pt_pool,
                tc.tile_pool(name="pvsb", bufs=2) as pvsb_pool,
                tc.tile_pool(name="rec", bufs=4) as rec_pool,
            ):
                for hp in range(8):
                    pv = pv_pool.tile([65, 2, 512], f32)
                    for g in range(8):
                        sc = sc_pool.tile([128, 2, 2, 512], f32)
                        for j in range(2):
                            kc = 2 * g + j
                            ks = slice(kc * 128, (kc + 1) * 128)
                            # adjacent MMs at base-partition 0/64 row-pack
                            nc.tensor.matmul(
                                sc[:, 0, j, :],
                                lhsT=kt_sb[0:64, hp, ks],
                                rhs=qeT[0:64, hp, :],
                                start=True,
                                stop=True,
                            )
                            nc.tensor.matmul(
                                sc[:, 1, j, :],
                                lhsT=kt_sb[64:128, hp, ks],
                                rhs=qeT[64:128, hp, :],
                                start=True,
                                stop=True,
                            )
                        pt = pt_pool.tile([128, 2, 2, 512], bf16)
                        nc.scalar.activation(
                            pt[:].rearrange("p a b c -> p (a b c)"),
                            sc[:].rearrange("p a b c -> p (a b c)"),
                            AF.Exp,
                        )
                        for j in range(2):
                            kc = 2 * g + j
                            for e in range(2):
                                nc.tensor.matmul(
                                    pv[:, e, :],
                                    lhsT=vsb[:, kc, 2 * hp + e, :],
                                    rhs=pt[:, e, j, :],
                                    start=(g == 0 and j == 0),
                                    stop=(g == 7 and j == 1),
                                )
                    pvsb = pvsb_pool.tile([65, 2, 512], f32)
                    nc.vector.tensor_copy(
                        pvsb[:].rearrange("p a b -> p (a b)"),
                        pv[:].rearrange("p a b -> p (a b)"),
                    )
                    for e in range(2):
                        h = 2 * hp + e
                        for qs in range(4):
                            tp2 = tp2_pool.tile([128, 65], f32)
                            nc.tensor.transpose(
                                tp2[:],
                                pvsb[:, e, qs * 128 : (qs + 1) * 128],
                                id_f32[0:65, 0:65],
                            )
                            rec = rec_pool.tile([128, 1], f32)
                            nc.vector.reciprocal(rec[:], tp2[:, 64:65])
                            nc.vector.tensor_scalar_mul(
                                outsb[qs][:, h * HD : (h + 1) * HD],
                                in0=tp2[:, 0:64],
                                scalar1=rec[:],
                            )
                for qs in range(4):
                    nc.vector.tensor_mul(outsb[qs][:], outsb[qs][:], vs_r[:])
                    nc.sync.dma_start(
                        out=out_d[qs * 128 : (qs + 1) * 128, :], in_=outsb[qs][:]
                    )

    nc.compile()
    return nc


def _host_prep(query, key, value, qs, ks_p, vs, vq_w, vq_b, ql_w, ql_b, ln_g, ln_b):
    """Fold the gate-parameter math on host; build per-core device inputs."""
    bf16 = ml_dtypes.bfloat16

    def sig(x):
        return 1.0 / (1.0 + np.exp(-x.astype(np.float64)))

    qsig = sig(qs).reshape(H)
    ksig = sig(ks_p).reshape(H)
    hg = sig(vs).reshape(H) @ vq_w.astype(np.float64).T + vq_b.astype(np.float64)
    c, f = hg[:H], hg[H:]
    vsig = (1.0 / (1.0 + np.exp(-f))) * np.tanh(c)
    gg = qsig * ksig / SCALE
    G = (gg * ln_g.astype(np.float64)).astype(np.float32)
    Bv = (gg * ln_b.astype(np.float64)).astype(np.float32)
    vsig = vsig.astype(np.float32)
    qlb = ql_b.astype(np.float32)

    wt_bf = np.ascontiguousarray(ql_w.astype(bf16).T)  # [2H, H]

    per_batch = {}
    for b in range(B):
        kt_bf = np.ascontiguousarray(key[:, b, :].astype(bf16).T)  # [H, S]
        v_b = value[:, b, :].reshape(NKC, 128, NH, HD)
        vaug = np.ascontiguousarray(
            np.concatenate(
                [v_b, np.ones((NKC, 128, NH, 1), np.float32)], axis=-1
            ).astype(bf16)
        )
        per_batch[b] = (kt_bf, vaug)

    in_maps = []
    for core in range(8):
        b, qc = core // 4, core % 4
        qt_bf = np.ascontiguousarray(
            query[qc * TQ : (qc + 1) * TQ, b, :].astype(bf16).T
        )  # [2H, TQ]
        kt_bf, vaug = per_batch[b]
        in_maps.append(
            {
                "qt": qt_bf,
                "kt": kt_bf,
                "wt": wt_bf,
                "vaug": vaug,
                "qlb": qlb,
                "gvec": G,
                "bvec": Bv,
                "vsig": vsig,
            }
        )
    return in_maps


def kernel(**inputs):
    from concourse.bass_utils import run_bass_kernel_spmd

    if "nc" not in _CACHE:
        _CACHE["nc"] = _build_bass()
    nc = _CACHE["nc"]

    in_maps = _host_prep(**inputs)
    res = run_bass_kernel_spmd(nc, in_maps, core_ids=list(range(8)))

    out = np.empty((S, B, H), np.float32)
    for core in range(8):
        b, qc = core // 4, core % 4
        out[qc * TQ : (qc + 1) * TQ, b, :] = res.results[core]["out"]
    return out


# revision 10
# speedup vs baseline: 1.6131x; 1.6131x over previous
"""Trainium2 Bass kernel for the gated-attention module (8 NeuronCores, SPMD).

Module math (per reference):
    qsig = sigmoid(qs); ksig = sigmoid(ks_p)
    vsig = sigmoid(f)*tanh(c),  (c,f) = split(sigmoid(vs) @ vq_w.T + vq_b)
    q = qsig * LN(query @ ql_w.T + ql_b)        [S,B,H]
    k = ksig * key ; v = vsig * value
    out[q,b,:] = softmax(q_h . k_h / sqrt(H)) @ v_h   (per head h)

Kernel strategy:
  - Shard (batch, query-block): core = b*4 + qc handles query rows
    [qc*512:(qc+1)*512] of batch b, with full K/V for that batch.
  - Host-side constant folding of the tiny gate vectors (pure functions of
    the module *parameters*, no data dependence):
        G  = qsig*ksig*ln_g/sqrt(H); Bv = qsig*ksig*ln_b/sqrt(H); vsig
    so on-device  q_eff = norm(y)*G + Bv,  scores = q_eff . key  (no key
    gating needed),  out = vsig * (P @ value).
  - bf16 matmul operands, pre-transposed on host into the contraction
    layouts the PE needs (q^T, k^T, w^T); fp32 psum accumulation for the
    q_linear and PV reductions; fp32 LN statistics and output.
  - Scores are computed transposed (k on partitions) so softmax's P feeds
    the PV matmul directly with no P transpose; the softmax denominator
    comes from a ones-column appended to V. exp() needs no max-subtract:
    |scores| <~ 0.4 (LN output scaled by sigmoid-gates/32), far from
    overflow.
  - Heads are processed in pairs with score matmuls interleaved at lhsT
    base-partitions 0/64 so the PE row-groups run them concurrently
    (contract dim is only 64).
"""

import sys

sys.path.insert(0, "/opt/trn_rl_repo")

import numpy as np
import ml_dtypes

S = 2048
B = 2
H = 1024
H2 = 2 * H
NH = 16
HD = 64
TQ = S // 4  # 512 query rows per core
NKC = S // 128  # 16 k-chunks
SCALE = float(np.sqrt(H))
EPS = 1e-12

_CACHE = {}


def _build_bass():
    import concourse.bacc as bacc
    import concourse.bass as bass
    import concourse.tile as tile
    from concourse import mybir
    from concourse.masks import make_identity

    f32 = mybir.dt.float32
    bf16 = mybir.dt.bfloat16
    AF = mybir.ActivationFunctionType
    ALU = mybir.AluOpType

    nc = bacc.Bacc(None, target_bir_lowering=False)

    qt_d = nc.dram_tensor("qt", [H2, TQ], bf16, kind="ExternalInput")
    kt_d = nc.dram_tensor("kt", [H, S], bf16, kind="ExternalInput")
    wt_d = nc.dram_tensor("wt", [H2, H], bf16, kind="ExternalInput")
    v_d = nc.dram_tensor("vaug", [NKC, 128, NH, HD + 1], bf16, kind="ExternalInput")
    qlb_d = nc.dram_tensor("qlb", [H], f32, kind="ExternalInput")
    g_d = nc.dram_tensor("gvec", [H], f32, kind="ExternalInput")
    bv_d = nc.dram_tensor("bvec", [H], f32, kind="ExternalInput")
    vs_d = nc.dram_tensor("vsig", [H], f32, kind="ExternalInput")
    out_d = nc.dram_tensor("out", [TQ, H], f32, kind="ExternalOutput")

    def bcast(dram_handle):
        # replicate a [H] dram vector across all 128 partitions
        ap = dram_handle[:]
        return bass.AP(tensor=ap.tensor, offset=ap.offset, ap=[[0, 128], [1, H]])

    with tile.TileContext(nc) as tc:
        with tc.tile_pool(name="persist", bufs=1) as persist:
            id_bf = persist.tile([128, 128], bf16)
            make_identity(nc, id_bf)
            id_f32 = persist.tile([128, 128], f32)
            make_identity(nc, id_f32)
            eps_t = persist.tile([128, 1], f32)
            nc.vector.memset(eps_t[:], EPS)

            qlb_r = persist.tile([128, H], f32)
            g_r = persist.tile([128, H], f32)
            bv_r = persist.tile([128, H], f32)
            vs_r = persist.tile([128, H], f32)
            nc.gpsimd.dma_start(out=qlb_r[:], in_=bcast(qlb_d))
            nc.gpsimd.dma_start(out=g_r[:], in_=bcast(g_d))
            nc.gpsimd.dma_start(out=bv_r[:], in_=bcast(bv_d))
            nc.gpsimd.dma_start(out=vs_r[:], in_=bcast(vs_d))

            # K^T tiles: kt_sb[p, dc, :] = key[:, dc*128+p] (host pre-transposed)
            kt_sb = persist.tile([128, 8, S], bf16)
            for half in range(2):
                nc.gpsimd.dma_start(
                    out=kt_sb[:, half * 4 : (half + 1) * 4, :],
                    in_=kt_d[half * 512 : (half + 1) * 512, :].rearrange(
                        "(dc p) k -> p dc k", p=128
                    ),
                )

            # V (+ ones column): vsb[p, kc, h, m] = vaug[kc, p, h, m]
            vsb = persist.tile([128, NKC, NH, HD + 1], bf16)
            nc.gpsimd.dma_start(out=vsb[:], in_=v_d[:].rearrange("c p h m -> p c h m"))

            # q_eff^T lives here: [o partitions, o-chunk, t]
            qeT = persist.tile([128, 8, TQ], bf16)
            # final output staging, one tile per 128-row query block
            outsb = [
                persist.tile([128, H], f32, name=f"outsb{i}", tag=f"outsb{i}")
                for i in range(4)
            ]

            # ---------------- phase 1+2: q_linear + LayerNorm ----------------
            with tc.tile_pool(name="ph2", bufs=1) as ph2:
                qt_sb = ph2.tile([128, 16, TQ], bf16)
                wt_sb = ph2.tile([128, 16, H], bf16)
                for g4 in range(4):
                    nc.sync.dma_start(
                        out=qt_sb[:, g4 * 4 : (g4 + 1) * 4, :],
                        in_=qt_d[g4 * 512 : (g4 + 1) * 512, :].rearrange(
                            "(ic p) t -> p ic t", p=128
                        ),
                    )
                    nc.sync.dma_start(
                        out=wt_sb[:, g4 * 4 : (g4 + 1) * 4, :],
                        in_=wt_d[g4 * 512 : (g4 + 1) * 512, :].rearrange(
                            "(ic p) o -> p ic o", p=128
                        ),
                    )
                ysb = [
                    ph2.tile([128, H], f32, name=f"ysb{i}", tag=f"ysb{i}")
                    for i in range(4)
                ]
                mv = [
                    ph2.tile([128, 2], f32, name=f"mv{i}", tag=f"mv{i}")
                    for i in range(4)
                ]
                rst = [
                    ph2.tile([128, 1], f32, name=f"rst{i}", tag=f"rst{i}")
                    for i in range(4)
                ]

                with (
                    tc.tile_pool(name="ylin", bufs=3, space="PSUM") as ylin,
                    tc.tile_pool(name="tpq", bufs=2, space="PSUM") as tpq,
                    tc.tile_pool(name="st", bufs=4) as st_pool,
                    tc.tile_pool(name="qe", bufs=4) as qe_pool,
                ):
                    # all q_linear matmuls back-to-back on the PE; LN chains
                    # (DVE/ACT) trail behind each chunk's eviction
                    for tc4 in range(4):
                        y_ps = ylin.tile([128, 2, 512], f32)
                        for ic in range(16):
                            lhsT = qt_sb[:, ic, tc4 * 128 : (tc4 + 1) * 128]
                            for oc in range(2):
                                nc.tensor.matmul(
                                    y_ps[:, oc, :],
                                    lhsT=lhsT,
                                    rhs=wt_sb[:, ic, oc * 512 : (oc + 1) * 512],
                                    start=(ic == 0),
                                    stop=(ic == 15),
                                )
                        nc.vector.tensor_add(
                            ysb[tc4][:],
                            y_ps[:].rearrange("p a b -> p (a b)"),
                            qlb_r[:],
                        )
                        st = st_pool.tile([128, 2, 6], f32)
                        nc.vector.bn_stats(st[:, 0, :], ysb[tc4][:, 0:512])
                        nc.vector.bn_stats(st[:, 1, :], ysb[tc4][:, 512:1024])
                        nc.vector.bn_aggr(mv[tc4][:], st[:])
                    # batched rstd: all Ln, then all Exp (one ACT table set)
                    lv = [
                        st_pool.tile([128, 1], f32, name=f"lv{i}", tag=f"lv{i}")
                        for i in range(4)
                    ]
                    for tc4 in range(4):
                        nc.scalar.activation(
                            lv[tc4][:], mv[tc4][:, 1:2], AF.Ln, bias=eps_t[:]
                        )
                    for tc4 in range(4):
                        nc.scalar.activation(
                            rst[tc4][:], lv[tc4][:], AF.Exp, scale=-0.5
                        )
                    qe = []
                    for tc4 in range(4):
                        nc.vector.tensor_scalar(
                            out=ysb[tc4][:],
                            in0=ysb[tc4][:],
                            scalar1=mv[tc4][:, 0:1],
                            scalar2=rst[tc4][:],
                            op0=ALU.subtract,
                            op1=ALU.mult,
                        )
                        nc.vector.tensor_mul(ysb[tc4][:], ysb[tc4][:], g_r[:])
                        q = qe_pool.tile([128, H], bf16, name=f"qe{tc4}")
                        nc.vector.tensor_add(q[:], ysb[tc4][:], bv_r[:])
                        qe.append(q)
                    # o-chunk-major transposes: head pair 0's q_eff^T finishes
                    # first so attention can begin while later chunks transpose
                    for oc8 in range(8):
                        for tc4 in range(4):
                            tp = tpq.tile([128, 128], bf16)
                            nc.tensor.transpose(
                                tp[:],
                                qe[tc4][:, oc8 * 128 : (oc8 + 1) * 128],
                                id_bf[:],
                            )
                            nc.vector.tensor_copy(
                                qeT[:, oc8, tc4 * 128 : (tc4 + 1) * 128], tp[:]
                            )

            # ---------------- phase 3: attention, head pairs ----------------
            with (
                tc.tile_pool(name="sc", bufs=2, space="PSUM") as sc_pool,
                tc.tile_pool(name="pv", bufs=1, space="PSUM") as pv_pool,
                tc.tile_pool(name="tp2", bufs=2, space="PSUM") as tp2_pool,
                tc.tile_pool(name="pt", bufs=3) as pt_pool,
                tc.tile_pool(name="pvsb", bufs=2) as pvsb_pool,
                tc.tile_pool(name="rec", bufs=4) as rec_pool,
            ):
                for hp in range(8):
                    pv = pv_pool.tile([65, 2, 512], f32)
                    for kc in range(NKC):
                        ks = slice(kc * 128, (kc + 1) * 128)
                        sc = sc_pool.tile([128, 2, 512], f32)
                        # adjacent MMs at base-partition 0/64 row-pack
                        nc.tensor.matmul(
                            sc[:, 0, :],
                            lhsT=kt_sb[0:64, hp, ks],
                            rhs=qeT[0:64, hp, :],
                            start=True,
                            stop=True,
                        )
                        nc.tensor.matmul(
                            sc[:, 1, :],
                            lhsT=kt_sb[64:128, hp, ks],
                            rhs=qeT[64:128, hp, :],
                            start=True,
                            stop=True,
                        )
                        pt = pt_pool.tile([128, 2, 512], bf16)
                        nc.scalar.activation(
                            pt[:].rearrange("p a b -> p (a b)"),
                            sc[:].rearrange("p a b -> p (a b)"),
                            AF.Exp,
                        )
                        for e in range(2):
                            nc.tensor.matmul(
                                pv[:, e, :],
                                lhsT=vsb[:, kc, 2 * hp + e, :],
                                rhs=pt[:, e, :],
                                start=(kc == 0),
                                stop=(kc == NKC - 1),
                            )
                    pvsb = pvsb_pool.tile([65, 2, 512], f32)
                    nc.vector.tensor_copy(
                        pvsb[:].rearrange("p a b -> p (a b)"),
                        pv[:].rearrange("p a b -> p (a b)"),
                    )
                    for e in range(2):
                        h = 2 * hp + e
                        for qs in range(4):
                            tp2 = tp2_pool.tile([128, 65], f32)
                            nc.tensor.transpose(
                                tp2[:],
                                pvsb[:, e, qs * 128 : (qs + 1) * 128],
                                id_f32[0:65, 0:65],
                            )
                            rec = rec_pool.tile([128, 1], f32)
                            nc.vector.reciprocal(rec[:], tp2[:, 64:65])
                            nc.vector.tensor_scalar_mul(
                                outsb[qs][:, h * HD : (h + 1) * HD],
                                in0=tp2[:, 0:64],
                                scalar1=rec[:],
                            )
                for qs in range(4):
                    nc.vector.tensor_mul(outsb[qs][:], outsb[qs][:], vs_r[:])
                    nc.sync.dma_start(
                        out=out_d[qs * 128 : (qs + 1) * 128, :], in_=outsb[qs][:]
                    )

    nc.compile()
    return nc


def _host_prep(query, key, value, qs, ks_p, vs, vq_w, vq_b, ql_w, ql_b, ln_g, ln_b):
    """Fold the gate-parameter math on host; build per-core device inputs."""
    bf16 = ml_dtypes.bfloat16

    def sig(x):
        return 1.0 / (1.0 + np.exp(-x.astype(np.float64)))

    qsig = sig(qs).reshape(H)
    ksig = sig(ks_p).reshape(H)
    hg = sig(vs).reshape(H) @ vq_w.astype(np.float64).T + vq_b.astype(np.float64)
    c, f = hg[:H], hg[H:]
    vsig = (1.0 / (1.0 + np.exp(-f))) * np.tanh(c)
    gg = qsig * ksig / SCALE
    G = (gg * ln_g.astype(np.float64)).astype(np.float32)
    Bv = (gg * ln_b.astype(np.float64)).astype(np.float32)
    vsig = vsig.astype(np.float32)
    qlb = ql_b.astype(np.float32)

    wt_bf = np.ascontiguousarray(ql_w.astype(bf16).T)  # [2H, H]

    per_batch = {}
    for b in range(B):
        kt_bf = np.ascontiguousarray(key[:, b, :].astype(bf16).T)  # [H, S]
        v_b = value[:, b, :].reshape(NKC, 128, NH, HD)
        vaug = np.ascontiguousarray(
            np.concatenate(
                [v_b, np.ones((NKC, 128, NH, 1), np.float32)], axis=-1
            ).astype(bf16)
        )
        per_batch[b] = (kt_bf, vaug)

    in_maps = []
    for core in range(8):
        b, qc = core // 4, core % 4
        qt_bf = np.ascontiguousarray(
            query[qc * TQ : (qc + 1) * TQ, b, :].astype(bf16).T
        )  # [2H, TQ]
        kt_bf, vaug = per_batch[b]
        in_maps.append(
            {
                "qt": qt_bf,
                "kt": kt_bf,
                "wt": wt_bf,
                "vaug": vaug,
                "qlb": qlb,
                "gvec": G,
                "bvec": Bv,
                "vsig": vsig,
            }
        )
    return in_maps


def kernel(**inputs):
    from concourse.bass_utils import run_bass_kernel_spmd

    if "nc" not in _CACHE:
        _CACHE["nc"] = _build_bass()
    nc = _CACHE["nc"]

    in_maps = _host_prep(**inputs)
    res = run_bass_kernel_spmd(nc, in_maps, core_ids=list(range(8)))

    out = np.empty((S, B, H), np.float32)
    for core in range(8):
        b, qc = core // 4, core % 4
        out[qc * TQ : (qc + 1) * TQ, b, :] = res.results[core]["out"]
    return out


# revision 12
# speedup vs baseline: 1.7403x; 1.0789x over previous
"""Trainium2 Bass kernel for the gated-attention module (8 NeuronCores, SPMD).

Module math (per reference):
    qsig = sigmoid(qs); ksig = sigmoid(ks_p)
    vsig = sigmoid(f)*tanh(c),  (c,f) = split(sigmoid(vs) @ vq_w.T + vq_b)
    q = qsig * LN(query @ ql_w.T + ql_b)        [S,B,H]
    k = ksig * key ; v = vsig * value
    out[q,b,:] = softmax(q_h . k_h / sqrt(H)) @ v_h   (per head h)

Kernel strategy:
  - Shard (batch, query-block): core = b*4 + qc handles query rows
    [qc*512:(qc+1)*512] of batch b, with full K/V for that batch.
  - Host-side constant folding of the tiny gate vectors (pure functions of
    the module *parameters*, no data dependence):
        G  = qsig*ksig*ln_g/sqrt(H); Bv = qsig*ksig*ln_b/sqrt(H); vsig
    so on-device  q_eff = norm(y)*G + Bv,  scores = q_eff . key  (no key
    gating needed),  out = vsig * (P @ value).
  - bf16 matmul operands, pre-transposed on host into the contraction
    layouts the PE needs (q^T, k^T, w^T); fp32 psum accumulation for the
    q_linear and PV reductions; fp32 LN statistics and output.
  - Scores are computed transposed (k on partitions) so softmax's P feeds
    the PV matmul directly with no P transpose; the softmax denominator
    comes from a ones-column appended to V. exp() needs no max-subtract:
    |scores| <~ 0.4 (LN output scaled by sigmoid-gates/32), far from
    overflow.
  - Heads are processed in pairs with score matmuls interleaved at lhsT
    base-partitions 0/64 so the PE row-groups run them concurrently
    (contract dim is only 64).
"""

import sys

sys.path.insert(0, "/opt/trn_rl_repo")

import numpy as np
import ml_dtypes

S = 2048
B = 2
H = 1024
H2 = 2 * H
NH = 16
HD = 64
TQ = S // 4  # 512 query rows per core
NKC = S // 128  # 16 k-chunks
SCALE = float(np.sqrt(H))
EPS = 1e-12

_CACHE = {}


def _build_bass():
    import concourse.bacc as bacc
    import concourse.bass as bass
    import concourse.tile as tile
    from concourse import mybir
    from concourse.masks import make_identity

    f32 = mybir.dt.float32
    bf16 = mybir.dt.bfloat16
    AF = mybir.ActivationFunctionType
    ALU = mybir.AluOpType

    nc = bacc.Bacc(None, target_bir_lowering=False)

    qt_d = nc.dram_tensor("qt", [H2, TQ], bf16, kind="ExternalInput")
    kt_d = nc.dram_tensor("kt", [H, S], bf16, kind="ExternalInput")
    wt_d = nc.dram_tensor("wt", [H2, H], bf16, kind="ExternalInput")
    v_d = nc.dram_tensor("vaug", [NKC, 128, NH, HD + 1], bf16, kind="ExternalInput")
    qlb_d = nc.dram_tensor("qlb", [H], f32, kind="ExternalInput")
    g_d = nc.dram_tensor("gvec", [H], f32, kind="ExternalInput")
    bv_d = nc.dram_tensor("bvec", [H], f32, kind="ExternalInput")
    vs_d = nc.dram_tensor("vsig", [H], f32, kind="ExternalInput")
    out_d = nc.dram_tensor("out", [TQ, H], f32, kind="ExternalOutput")

    def bcast(dram_handle):
        # replicate a [H] dram vector across all 128 partitions
        ap = dram_handle[:]
        return bass.AP(tensor=ap.tensor, offset=ap.offset, ap=[[0, 128], [1, H]])

    with tile.TileContext(nc) as tc:
        with tc.tile_pool(name="persist", bufs=1) as persist:
            id_bf = persist.tile([128, 128], bf16)
            make_identity(nc, id_bf)
            id_f32 = persist.tile([128, 128], f32)
            make_identity(nc, id_f32)
            eps_t = persist.tile([128, 1], f32)
            nc.vector.memset(eps_t[:], EPS)

            qlb_r = persist.tile([128, H], f32)
            g_r = persist.tile([128, H], f32)
            bv_r = persist.tile([128, H], f32)
            vs_r = persist.tile([128, H], f32)
            nc.gpsimd.dma_start(out=qlb_r[:], in_=bcast(qlb_d))
            nc.gpsimd.dma_start(out=g_r[:], in_=bcast(g_d))
            nc.gpsimd.dma_start(out=bv_r[:], in_=bcast(bv_d))
            nc.gpsimd.dma_start(out=vs_r[:], in_=bcast(vs_d))

            # K^T tiles: kt_sb[p, dc, :] = key[:, dc*128+p] (host pre-transposed)
            kt_sb = persist.tile([128, 8, S], bf16)
            # V (+ ones column): vsb[p, kc, h, m] = vaug[kc, p, h, m]
            vsb = persist.tile([128, NKC, NH, HD + 1], bf16)

            # q_eff^T lives here: [o partitions, o-chunk, t]
            qeT = persist.tile([128, 8, TQ], bf16)
            # final output staging, one tile per 128-row query block
            outsb = [
                persist.tile([128, H], f32, name=f"outsb{i}", tag=f"outsb{i}")
                for i in range(4)
            ]

            # ---------------- phase 1+2: q_linear + LayerNorm ----------------
            with tc.tile_pool(name="ph2", bufs=1) as ph2:
                qt_sb = ph2.tile([128, 16, TQ], bf16)
                wt_sb = ph2.tile([128, 16, H], bf16)
                # q^T chunks on the Sync HWDGE queue, w^T on the Scalar HWDGE
                # queue so both streams load in parallel and the q_linear
                # matmuls can chase them chunk by chunk
                for g4 in range(4):
                    nc.sync.dma_start(
                        out=qt_sb[:, g4 * 4 : (g4 + 1) * 4, :],
                        in_=qt_d[g4 * 512 : (g4 + 1) * 512, :].rearrange(
                            "(ic p) t -> p ic t", p=128
                        ),
                    )
                    nc.scalar.dma_start(
                        out=wt_sb[:, g4 * 4 : (g4 + 1) * 4, :],
                        in_=wt_d[g4 * 512 : (g4 + 1) * 512, :].rearrange(
                            "(ic p) o -> p ic o", p=128
                        ),
                    )
                # phase-3 operands load behind them on the same HWDGE queues
                for half in range(2):
                    nc.sync.dma_start(
                        out=kt_sb[:, half * 4 : (half + 1) * 4, :],
                        in_=kt_d[half * 512 : (half + 1) * 512, :].rearrange(
                            "(dc p) k -> p dc k", p=128
                        ),
                    )
                nc.scalar.dma_start(
                    out=vsb[:], in_=v_d[:].rearrange("c p h m -> p c h m")
                )
                ysb = [
                    ph2.tile([128, H], f32, name=f"ysb{i}", tag=f"ysb{i}")
                    for i in range(4)
                ]
                mv = [
                    ph2.tile([128, 2], f32, name=f"mv{i}", tag=f"mv{i}")
                    for i in range(4)
                ]
                rst = [
                    ph2.tile([128, 1], f32, name=f"rst{i}", tag=f"rst{i}")
                    for i in range(4)
                ]

                with (
                    tc.tile_pool(name="ylin", bufs=3, space="PSUM") as ylin,
                    tc.tile_pool(name="tpq", bufs=2, space="PSUM") as tpq,
                    tc.tile_pool(name="st", bufs=4) as st_pool,
                    tc.tile_pool(name="qe", bufs=4) as qe_pool,
                ):
                    # all q_linear matmuls back-to-back on the PE; LN chains
                    # (DVE/ACT) trail behind each chunk's eviction
                    for tc4 in range(4):
                        y_ps = ylin.tile([128, 2, 512], f32)
                        for ic in range(16):
                            lhsT = qt_sb[:, ic, tc4 * 128 : (tc4 + 1) * 128]
                            for oc in range(2):
                                nc.tensor.matmul(
                                    y_ps[:, oc, :],
                                    lhsT=lhsT,
                                    rhs=wt_sb[:, ic, oc * 512 : (oc + 1) * 512],
                                    start=(ic == 0),
                                    stop=(ic == 15),
                                )
                        nc.vector.tensor_add(
                            ysb[tc4][:],
                            y_ps[:].rearrange("p a b -> p (a b)"),
                            qlb_r[:],
                        )
                        st = st_pool.tile([128, 2, 6], f32)
                        nc.vector.bn_stats(st[:, 0, :], ysb[tc4][:, 0:512])
                        nc.vector.bn_stats(st[:, 1, :], ysb[tc4][:, 512:1024])
                        nc.vector.bn_aggr(mv[tc4][:], st[:])
                    # batched rstd: all Ln, then all Exp (one ACT table set)
                    lv = [
                        st_pool.tile([128, 1], f32, name=f"lv{i}", tag=f"lv{i}")
                        for i in range(4)
                    ]
                    for tc4 in range(4):
                        nc.scalar.activation(
                            lv[tc4][:], mv[tc4][:, 1:2], AF.Ln, bias=eps_t[:]
                        )
                    for tc4 in range(4):
                        nc.scalar.activation(
                            rst[tc4][:], lv[tc4][:], AF.Exp, scale=-0.5
                        )
                    qe = []
                    for tc4 in range(4):
                        nc.vector.tensor_scalar(
                            out=ysb[tc4][:],
                            in0=ysb[tc4][:],
                            scalar1=mv[tc4][:, 0:1],
                            scalar2=rst[tc4][:],
                            op0=ALU.subtract,
                            op1=ALU.mult,
                        )
                        nc.vector.tensor_mul(ysb[tc4][:], ysb[tc4][:], g_r[:])
                        q = qe_pool.tile([128, H], bf16, name=f"qe{tc4}")
                        nc.vector.tensor_add(q[:], ysb[tc4][:], bv_r[:])
                        qe.append(q)
                    # o-chunk-major transposes: head pair 0's q_eff^T finishes
                    # first so attention can begin while later chunks transpose
                    for oc8 in range(8):
                        for tc4 in range(4):
                            tp = tpq.tile([128, 128], bf16)
                            nc.tensor.transpose(
                                tp[:],
                                qe[tc4][:, oc8 * 128 : (oc8 + 1) * 128],
                                id_bf[:],
                            )
                            nc.vector.tensor_copy(
                                qeT[:, oc8, tc4 * 128 : (tc4 + 1) * 128], tp[:]
                            )

            # ---------------- phase 3: attention, head pairs ----------------
            with (
                tc.tile_pool(name="sc", bufs=2, space="PSUM") as sc_pool,
                tc.tile_pool(name="pv", bufs=1, space="PSUM") as pv_pool,
                tc.tile_pool(name="tp2", bufs=2, space="PSUM") as tp2_pool,
                tc.tile_pool(name="pt", bufs=3) as pt_pool,
                tc.tile_pool(name="pvsb", bufs=2) as pvsb_pool,
                tc.tile_pool(name="rec", bufs=4) as rec_pool,
            ):
                for hp in range(8):
                    pv = pv_pool.tile([65, 2, 512], f32)
                    for kc in range(NKC):
                        ks = slice(kc * 128, (kc + 1) * 128)
                        sc = sc_pool.tile([128, 2, 512], f32)
                        # adjacent MMs at base-partition 0/64 row-pack
                        nc.tensor.matmul(
                            sc[:, 0, :],
                            lhsT=kt_sb[0:64, hp, ks],
                            rhs=qeT[0:64, hp, :],
                            start=True,
                            stop=True,
                        )
                        nc.tensor.matmul(
                            sc[:, 1, :],
                            lhsT=kt_sb[64:128, hp, ks],
                            rhs=qeT[64:128, hp, :],
                            start=True,
                            stop=True,
                        )
                        pt = pt_pool.tile([128, 2, 512], bf16)
                        nc.scalar.activation(
                            pt[:].rearrange("p a b -> p (a b)"),
                            sc[:].rearrange("p a b -> p (a b)"),
                            AF.Exp,
                        )
                        for e in range(2):
                            nc.tensor.matmul(
                                pv[:, e, :],
                                lhsT=vsb[:, kc, 2 * hp + e, :],
                                rhs=pt[:, e, :],
                                start=(kc == 0),
                                stop=(kc == NKC - 1),
                            )
                    pvsb = pvsb_pool.tile([65, 2, 512], f32)
                    nc.vector.tensor_copy(
                        pvsb[:].rearrange("p a b -> p (a b)"),
                        pv[:].rearrange("p a b -> p (a b)"),
                    )
                    for e in range(2):
                        h = 2 * hp + e
                        for qs in range(4):
                            tp2 = tp2_pool.tile([128, 65], f32)
                            nc.tensor.transpose(
                                tp2[:],
                                pvsb[:, e, qs * 128 : (qs + 1) * 128],
                                id_f32[0:65, 0:65],
                            )
                            rec = rec_pool.tile([128, 1], f32)
                            nc.vector.reciprocal(rec[:], tp2[:, 64:65])
                            nc.vector.tensor_scalar_mul(
                                outsb[qs][:, h * HD : (h + 1) * HD],
                                in0=tp2[:, 0:64],
                                scalar1=rec[:],
                            )
                for qs in range(4):
                    nc.vector.tensor_mul(outsb[qs][:], outsb[qs][:], vs_r[:])
                    nc.sync.dma_start(
                        out=out_d[qs * 128 : (qs + 1) * 128, :], in_=outsb[qs][:]
                    )

    nc.compile()
    return nc


def _host_prep(query, key, value, qs, ks_p, vs, vq_w, vq_b, ql_w, ql_b, ln_g, ln_b):
    """Fold the gate-parameter math on host; build per-core device inputs."""
    bf16 = ml_dtypes.bfloat16

    def sig(x):
        return 1.0 / (1.0 + np.exp(-x.astype(np.float64)))

    qsig = sig(qs).reshape(H)
    ksig = sig(ks_p).reshape(H)
    hg = sig(vs).reshape(H) @ vq_w.astype(np.float64).T + vq_b.astype(np.float64)
    c, f = hg[:H], hg[H:]
    vsig = (1.0 / (1.0 + np.exp(-f))) * np.tanh(c)
    gg = qsig * ksig / SCALE
    G = (gg * ln_g.astype(np.float64)).astype(np.float32)
    Bv = (gg * ln_b.astype(np.float64)).astype(np.float32)
    vsig = vsig.astype(np.float32)
    qlb = ql_b.astype(np.float32)

    wt_bf = np.ascontiguousarray(ql_w.astype(bf16).T)  # [2H, H]

    per_batch = {}
    for b in range(B):
        kt_bf = np.ascontiguousarray(key[:, b, :].astype(bf16).T)  # [H, S]
        v_b = value[:, b, :].reshape(NKC, 128, NH, HD)
        vaug = np.ascontiguousarray(
            np.concatenate(
                [v_b, np.ones((NKC, 128, NH, 1), np.float32)], axis=-1
            ).astype(bf16)
        )
        per_batch[b] = (kt_bf, vaug)

    in_maps = []
    for core in range(8):
        b, qc = core // 4, core % 4
        qt_bf = np.ascontiguousarray(
            query[qc * TQ : (qc + 1) * TQ, b, :].astype(bf16).T
        )  # [2H, TQ]
        kt_bf, vaug = per_batch[b]
        in_maps.append(
            {
                "qt": qt_bf,
                "kt": kt_bf,
                "wt": wt_bf,
                "vaug": vaug,
                "qlb": qlb,
                "gvec": G,
                "bvec": Bv,
                "vsig": vsig,
            }
        )
    return in_maps


def kernel(**inputs):
    from concourse.bass_utils import run_bass_kernel_spmd

    if "nc" not in _CACHE:
        _CACHE["nc"] = _build_bass()
    nc = _CACHE["nc"]

    in_maps = _host_prep(**inputs)
    res = run_bass_kernel_spmd(nc, in_maps, core_ids=list(range(8)))

    out = np.empty((S, B, H), np.float32)
    for core in range(8):
        b, qc = core // 4, core % 4
        out[qc * TQ : (qc + 1) * TQ, b, :] = res.results[core]["out"]
    return out
